# revision 30
# baseline (speedup 1.0000x reference)
"""Trainium2 Bass kernel for per-sample covariance pooling + FC + L2 normalize.

Reference computation (per sample of x [B=32, N=50000, D=64]):
    xc  = x - mean(x, axis=N)
    cov = xc^T xc / (N-1)               # [64, 64]
    out = cov.flatten() @ W.T + b       # [256]
    out = out / max(||out||_2, 1e-12)

Sharding: data-parallel over batch B across 8 NeuronCores (4 samples/core).
W (fed pre-transposed as [4096, 256]) and b are replicated.

Host-side marshalling appends a ones column to x and zero-pads rows to a
whole number of chunks (-> [B, NPAD, 65]); zero rows are inert for both
reductions.  The ones column lets a single accumulating matmul per
[128, 65] tile produce both S = X^T X (PSUM rows 0:64) and the column
sums s (row 64), while keeping every DMA fully contiguous on both sides
(260B rows; partition p holds a contiguous block of CHUNK_T rows — row
order is irrelevant to the S and s reductions).

Per-core algorithm:
  - x streams in 1.86MiB chunks on the SWDGE queue with an inline
    fp32->bf16 cast; W^T (fp16) and b ride the separate HWDGE ring, which
    fills idle gaps of the x-stream instead of serializing ahead of it.
  - 392 accumulating PE matmuls per sample (bf16, K=128, M=65, N=64),
    deep chunk prefetch (bufs=8) to ride out PE/HAM jitter; the final
    chunk is split in two so its matmuls overlap the transfer.
  - Mean correction: scale s on partition 64, then a K=1 outer-product
    matmul accumulates -(s/sqrt(N))(s/sqrt(N))^T into PSUM rows 0:64.
  - cov = PSUM * 1/(N-1) -> fp16 SBUF tile cov4[64, sample, 64].
  - FC uses cov symmetry: flat[64*t + e] = cov_s[t, e] = cov_s[e, t], so
    contraction tile t (K=64) is just the cov column slice cov4[:, :, t].
    64 accumulating fp16 matmuls against W^T tiles -> PSUM [4, 256].
  - bias add, L2 normalize (DVE/ACT), DMA out [4, 256] per core.
"""

import math
import numpy as np
from contextlib import ExitStack

import concourse.bass as bass
import concourse.tile as tile
from concourse import bacc, mybir
from concourse import bass_utils
from concourse._compat import with_exitstack

B, N_FULL, D, OUT = 32, 50000, 64, 256
DA = D + 1  # x augmented with a ones column
NCORES = 8
BPC = B // NCORES  # samples per core
P = 128  # partitions per n-tile
CHUNK_T = 56  # n-tiles per DMA chunk (128*56 rows = 1.86MiB fp32)

F32 = mybir.dt.float32
BF16 = mybir.dt.bfloat16
FC_DT = mybir.dt.float16  # FC runs at bf16 speed with 2^-11 rounding


@with_exitstack
def _cov_kernel(
    ctx: ExitStack,
    tc: tile.TileContext,
    out: bass.AP,
    xs: bass.AP,
    wt: bass.AP,
    b1: bass.AP,
    n_rows: int,
    n_true: int,
):
    nc = tc.nc
    n_chunks = n_rows // (CHUNK_T * P)
    assert n_chunks * CHUNK_T * P == n_rows, "n_rows must split into whole chunks"
    inv_sqrt_n = 1.0 / math.sqrt(n_true)
    inv_nm1 = 1.0 / (n_true - 1)

    xsf = xs.rearrange("b n e -> (b n) e")  # [BPC*n_rows, 65]

    chunks = ctx.enter_context(tc.tile_pool(name="chunks", bufs=8))
    smalls = ctx.enter_context(tc.tile_pool(name="smalls", bufs=4))
    singles = ctx.enter_context(tc.tile_pool(name="singles", bufs=1))
    psum_s = ctx.enter_context(tc.tile_pool(name="psum_s", bufs=2, space="PSUM"))
    psum_fc = ctx.enter_context(tc.tile_pool(name="psum_fc", bufs=2, space="PSUM"))

    # Replicated FC weights: W^T [4096, 256] fp16 (host precision choice for
    # the FC weight) -> tiles [e, t, o] where f = 64*t + e is the flattened
    # cov index.  Loaded on the HWDGE ring, which fills idle gaps of the
    # SWDGE x-stream instead of serializing ahead of it.
    wt_sb = singles.tile([64, 64, OUT], FC_DT)
    nc.sync.dma_start(out=wt_sb, in_=wt.rearrange("(t p) o -> p t o", p=64))
    b4_sb = singles.tile([BPC, OUT], F32)
    nc.sync.dma_start(out=b4_sb, in_=b1.to_broadcast([BPC, OUT]))

    # cov4[e, s, d] = cov_s[d, e] (symmetric, so also cov_s[e, d])
    cov4 = singles.tile([64, BPC, 64], FC_DT)

    # Preload the ScalarE Sqrt LUT during the stream so the tail's L2-norm
    # sqrt doesn't pay a lazy ~1.3us ACT_TABLE_LOAD on the critical path.
    sqwarm = singles.tile([1, 1], F32)
    nc.scalar.sqrt(sqwarm, b4_sb[0:1, 0:1])

    for s in range(BPC):
        ps = psum_s.tile([65, 64], F32)
        base = s * n_rows
        for c in range(n_chunks):
            # Partition p holds rows [p*CHUNK_T, (p+1)*CHUNK_T) of the chunk:
            # both DMA sides are contiguous per partition (big descriptors).
            # Progressively smaller pieces at the very end: only the last
            # piece's matmuls trail the final DMA byte.
            last = s == BPC - 1 and c == n_chunks - 1
            splits = [CHUNK_T // 2, CHUNK_T // 4, CHUNK_T // 8, CHUNK_T - CHUNK_T // 2 - CHUNK_T // 4 - CHUNK_T // 8] if last else [CHUNK_T]
            r0 = base + c * (CHUNK_T * P)
            first_mm = c == 0
            for j, tcnt in enumerate(splits):
                ctile = chunks.tile([P, tcnt, DA], BF16, tag="ctile")
                nc.gpsimd.dma_start(
                    out=ctile,
                    in_=xsf[r0 : r0 + tcnt * P, :].rearrange(
                        "(p q) e -> p q e", q=tcnt
                    ),
                )
                r0 += tcnt * P
                for q in range(tcnt):
                    nc.tensor.matmul(
                        ps,
                        lhsT=ctile[:, q, :],
                        rhs=ctile[:, q, 0:64],
                        start=(first_mm and q == 0),
                        stop=(
                            c == n_chunks - 1
                            and j == len(splits) - 1
                            and q == tcnt - 1
                        ),
                    )
                first_mm = False

        # Column sums s sit in PSUM row 64.  Scale into SBUF on the same
        # partition; the K=1 outer-product matmul runs from partition 64
        # (tile_position (64, 0)), accumulating -s s^T / N into rows 0:64.
        sboth = smalls.tile([65, 2, 64], F32)
        nc.scalar.mul(sboth[64:65, 0, :], ps[64:65, :], inv_sqrt_n)
        nc.scalar.mul(sboth[64:65, 1, :], ps[64:65, :], -inv_sqrt_n)
        nc.tensor.matmul(
            ps[0:64, :],
            lhsT=sboth[64:65, 0, :],
            rhs=sboth[64:65, 1, :],
            start=False,
            stop=True,
            skip_group_check=True,
        )
        nc.scalar.mul(out=cov4[:, s, :], in_=ps[0:64, :], mul=inv_nm1)

    # Joint FC for all samples (PE is in-order, so per-sample FC would stall
    # the stream on the lazily-loading wt): out[s, o] accumulates over the 64
    # K=64 contraction tiles; M=BPC, N=OUT, fp16 -> ~7us tail.
    po = psum_fc.tile([BPC, OUT], F32)
    for t in range(64):
        nc.tensor.matmul(
            po,
            lhsT=cov4[:, :, t],
            rhs=wt_sb[:, t, :],
            start=(t == 0),
            stop=(t == 63),
        )
    o_sb = smalls.tile([BPC, OUT], F32)
    nc.vector.tensor_add(o_sb, po, b4_sb)
    sq = smalls.tile([BPC, OUT], F32)
    nc.vector.tensor_mul(sq, o_sb, o_sb)
    ss = smalls.tile([BPC, 1], F32)
    nc.vector.reduce_sum(out=ss, in_=sq, axis=mybir.AxisListType.X)
    nrm = smalls.tile([BPC, 1], F32)
    nc.scalar.sqrt(nrm, ss)
    nc.vector.tensor_scalar_max(nrm, nrm, 1e-12)
    rn = smalls.tile([BPC, 1], F32)
    nc.vector.reciprocal(rn, nrm)
    nc.vector.tensor_scalar_mul(o_sb, o_sb, rn)
    nc.sync.dma_start(out=out, in_=o_sb)


def pad_rows(n: int) -> int:
    step = CHUNK_T * P
    return ((n + step - 1) // step) * step


def build(n_true: int = N_FULL, enable_asserts: bool = False):
    n_rows = pad_rows(n_true)
    nc = bacc.Bacc(
        "TRN2",
        target_bir_lowering=False,
        debug=False,
        enable_asserts=enable_asserts,
        num_devices=NCORES,
    )
    xs = nc.dram_tensor("xs", [BPC, n_rows, DA], F32, kind="ExternalInput").ap()
    wt = nc.dram_tensor("wt", [D * D, OUT], FC_DT, kind="ExternalInput").ap()
    b1 = nc.dram_tensor("b1", [1, OUT], F32, kind="ExternalInput").ap()
    out = nc.dram_tensor("out", [BPC, OUT], F32, kind="ExternalOutput").ap()
    with tile.TileContext(nc) as tc:
        _cov_kernel(tc, out, xs, wt, b1, n_rows, n_true)
    nc.compile()
    return nc


_cache: dict = {}


def make_in_maps(x: np.ndarray, W: np.ndarray, b: np.ndarray):
    # Append the ones column and zero-pad rows to whole chunks on the host
    # (zero rows contribute nothing to S or s; ones col is 0 there too).
    bb, nn, _ = x.shape
    npad = pad_rows(nn)
    xa = np.zeros((bb, npad, DA), dtype=np.float32)
    xa[:, :nn, :D] = x
    xa[:, :nn, D] = 1.0
    wt = np.ascontiguousarray(W.T.astype(np.float16))
    b1 = np.asarray(b, dtype=np.float32).reshape(1, OUT)
    return [
        {
            "xs": np.ascontiguousarray(xa[k * BPC : (k + 1) * BPC]),
            "wt": wt,
            "b1": b1,
        }
        for k in range(NCORES)
    ]


def kernel(x: np.ndarray, W: np.ndarray, b: np.ndarray, **run_kwargs) -> np.ndarray:
    x = np.asarray(x, dtype=np.float32)
    assert x.shape == (B, N_FULL, D), x.shape
    if "nc" not in _cache:
        _cache["nc"] = build(N_FULL)
    nc = _cache["nc"]
    res = bass_utils.run_bass_kernel_spmd(
        nc, make_in_maps(x, W, b), core_ids=list(range(NCORES)), **run_kwargs
    )
    out = np.concatenate([r["out"] for r in res.results], axis=0)
    _cache["last_results"] = res
    return out
